# revision 26
# baseline (speedup 1.0000x reference)
"""MiniMaxText01 linear attention layer on 8 trn2 NeuronCores.

Strategy: tensor-parallel over heads (4 heads/core), single fused pass in
bf16. Per 512-token chunk, entirely SBUF-resident (no DRAM staging):
  1. q/k/gate/v projections (bf16 matmuls, fp32 PSUM accum, silu/sigmoid
     on ACT writing bf16 staging). q/k/gate staged transposed [d, tok],
     v natural [tok, c].
  2. blocked lightning attention (BLOCK=256): scores = kT-slices @ q,
     decay+causal mask folded into one DVE multiply; k natural obtained
     via PE transpose (bf16); kv state [d, 4h, e] bf16 in SBUF, decay
     bd*kv folded into the PSUM accumulation as a (bd*I) matmul.
  3. gate multiply (DVE, from PSUM), squares for the RMSNorm variance
     (ACT), out projection (bf16) DMAed straight from PSUM to DRAM fp32.
norm_weight is folded into w_out rows on the host; the RMSNorm rsqrt is a
per-token scalar applied on the host after summing partial outputs:
  out = sum_c(pout_c) * rsqrt(sum_c(ssq_c)/4096 + eps).
"""
import numpy as np
from contextlib import ExitStack

import ml_dtypes
import concourse.bass as bass
import concourse.tile as tile
import concourse.mybir as mybir
from concourse import bacc
from concourse.bass_utils import run_bass_kernel_spmd

FP32 = mybir.dt.float32
BF16 = mybir.dt.bfloat16
AF = mybir.ActivationFunctionType
BF = ml_dtypes.bfloat16

SEQ = 4096
HIDDEN = 2048
NUM_HEADS = 32
HEAD_DIM = 128
INNER = NUM_HEADS * HEAD_DIM
BLOCK = 256
EPS = 1e-5
N_CORES = 8
HPC = NUM_HEADS // N_CORES          # 4 heads per core
IN_PC = HPC * HEAD_DIM              # 512 inner channels per core
P = 128

CH = 512                            # token chunk
NT = SEQ // CH                      # 8 chunks
HC = HIDDEN // P                    # 16 hidden chunks
NBLK = CH // BLOCK                  # 2 blocks per chunk


def build_nc(repeat: int = 1, phases: str = "AB", nA: int = 1, nB: int = 1):
    nc = bacc.Bacc("TRN2", target_bir_lowering=False)

    xt_d = nc.dram_tensor("xt", [HIDDEN, SEQ], BF16, kind="ExternalInput")
    wq_d = nc.dram_tensor("wq", [HIDDEN, IN_PC], BF16, kind="ExternalInput")
    wk_d = nc.dram_tensor("wk", [HIDDEN, IN_PC], BF16, kind="ExternalInput")
    wv_d = nc.dram_tensor("wv", [HIDDEN, IN_PC], BF16, kind="ExternalInput")
    wg_d = nc.dram_tensor("wg", [HIDDEN, IN_PC], BF16, kind="ExternalInput")
    wo_d = nc.dram_tensor("wo", [IN_PC, HIDDEN], BF16, kind="ExternalInput")
    qdec_d = nc.dram_tensor("qdec", [HPC, P, BLOCK], BF16, kind="ExternalInput")
    dmask0_d = nc.dram_tensor("dmask0", [HPC, P, BLOCK], BF16, kind="ExternalInput")
    dmask1_d = nc.dram_tensor("dmask1", [HPC, P, P], BF16, kind="ExternalInput")
    kdec_d = nc.dram_tensor("kdec", [HPC, P, BLOCK], BF16, kind="ExternalInput")
    bdi_d = nc.dram_tensor("bdi", [HPC, P, P], BF16, kind="ExternalInput")
    ones_d = nc.dram_tensor("ones", [P, 1], BF16, kind="ExternalInput")
    kv0_d = nc.dram_tensor("kv0", [HPC, P, P], BF16, kind="ExternalInput")

    pout_d = nc.dram_tensor("pout", [SEQ, HIDDEN], BF16, kind="ExternalOutput")
    ssq_d = nc.dram_tensor("ssq", [1, SEQ], FP32, kind="ExternalOutput")

    with tile.TileContext(nc) as tc, ExitStack() as ctx:
        const = ctx.enter_context(tc.tile_pool(name="const", bufs=1))
        wpool = ctx.enter_context(tc.tile_pool(name="w", bufs=1))
        xpool = ctx.enter_context(tc.tile_pool(name="x", bufs=1))
        spool = ctx.enter_context(tc.tile_pool(name="stg", bufs=1))
        psum = ctx.enter_context(tc.tile_pool(name="psum", bufs=1, space="PSUM"))

        # ---- weights first on the sync queue (q -> k -> g -> v), so the
        # first projection can start as soon as wq lands; consts + wo after.
        WG = 4                          # hc-groups per weight DMA
        wq_t = wpool.tile([P, HC, IN_PC], BF16)
        for g in range(HC // WG):
            nc.sync.dma_start(
                wq_t[:, g * WG:(g + 1) * WG, :],
                wq_d[g * WG * P:(g + 1) * WG * P, :]
                .rearrange("(hc p) m -> p hc m", p=P))
        wq_l = wq_t
        wk_t = wpool.tile([P, HC, IN_PC], BF16)
        wg_t = wpool.tile([P, HC, IN_PC], BF16)
        wv_t = wpool.tile([P, HC, IN_PC], BF16)
        for w_t, w_d in ((wk_t, wk_d), (wg_t, wg_d), (wv_t, wv_d)):
            for g in range(HC // WG):
                hsl = slice(g * WG * P, (g + 1) * WG * P)
                gsl = slice(g * WG, (g + 1) * WG)
                nc.sync.dma_start(
                    w_t[:, gsl, :],
                    w_d[hsl, :].rearrange("(hc p) m -> p hc m", p=P))

        ones_t = const.tile([P, 1], BF16)
        nc.sync.dma_start(ones_t[:], ones_d[:])
        qdec_t = const.tile([P, HPC, BLOCK], BF16)
        nc.sync.dma_start(qdec_t[:], qdec_d[:].rearrange("h p i -> p h i"))
        dmask0_t = const.tile([P, HPC, BLOCK], BF16)
        nc.sync.dma_start(dmask0_t[:], dmask0_d[:].rearrange("h p i -> p h i"))
        dmask1_t = const.tile([P, HPC, P], BF16)
        nc.sync.dma_start(dmask1_t[:], dmask1_d[:].rearrange("h p i -> p h i"))
        kdec_t = const.tile([P, HPC, BLOCK], BF16)
        nc.sync.dma_start(kdec_t[:], kdec_d[:].rearrange("h p i -> p h i"))
        bdi_t = const.tile([P, HPC, P], BF16)
        nc.sync.dma_start(bdi_t[:], bdi_d[:].rearrange("h d e -> d h e"))
        kv_t = const.tile([P, HPC, P], BF16)

        wo_t = wpool.tile([P, HPC, HIDDEN], BF16)
        nc.sync.dma_start(wo_t[:], wo_d[:].rearrange("(h p) n -> p h n", p=P))

        xt_r = xt_d[:].rearrange("(hc p) n -> p hc n", p=P)

        for _rep in range(repeat):
            nc.scalar.dma_start(kv_t[:], kv0_d[:].rearrange("h d e -> d h e"))

            for t in range(NT):
                tsl = slice(t * CH, (t + 1) * CH)

                # x chunk on the Activation HWDGE queue so weight/pout
                # traffic on the SP queue never delays it; 4 separate tiles
                # so early hc-slices are consumable while the rest stream in
                xs_l = []
                for g in range(4):
                    xg = xpool.tile([P, 4, CH], BF16, tag=f"x{g}", bufs=2)
                    nc.scalar.dma_start(xg[:], xt_r[:, g * 4:(g + 1) * 4, tsl])
                    xs_l.append(xg)

                def xsl(hc):
                    return xs_l[hc // 4][:, hc % 4, :]

                # ---- projections ----
                q_s = spool.tile([P, HPC, CH], BF16, tag="q", bufs=2)
                k_s = spool.tile([P, HPC, CH], BF16, tag="k", bufs=2)
                g_s = spool.tile([P, HPC, CH], BF16, tag="g", bufs=2)
                v_s = spool.tile([P, NBLK * 2, IN_PC], BF16, tag="v", bufs=2)

                def proj(w_l, actf, dst):
                    for cc in range(HPC):
                        ps = psum.tile([P, CH], FP32, tag="ps512", bufs=4)
                        for hc in range(HC):
                            w_t = w_l[hc // 4] if isinstance(w_l, list) else w_l
                            wi = hc % 4 if isinstance(w_l, list) else hc
                            nc.tensor.matmul(
                                ps[:], w_t[:, wi, cc * P:(cc + 1) * P],
                                xsl(hc),
                                start=(hc == 0), stop=(hc == HC - 1))
                        nc.scalar.activation(dst[:, cc, :], ps[:], actf)

                proj(wq_l, AF.Silu, q_s)
                proj(wk_t, AF.Silu, k_s)

                gA_s = spool.tile([P, HPC, CH], BF16, tag="gA", bufs=2)
                sq_s = spool.tile([P, HPC, CH], BF16, tag="sq", bufs=2)

                # k * k-decay (DVE), then k natural via XBAR DMA transpose —
                # emitted right after the k projection so the DMA latency
                # hides under the g/v projection matmuls.
                kd_s = spool.tile([P, HPC, CH], BF16, tag="kd", bufs=2)
                kn_l = []
                for b in range(NBLK):
                    t0 = b * BLOCK
                    for h in range(HPC):
                        nc.vector.tensor_mul(kd_s[:, h, t0:t0 + BLOCK],
                                             k_s[:, h, t0:t0 + BLOCK],
                                             kdec_t[:, h, :])
                    kn_s = spool.tile([P, 2 * HPC, P], BF16,
                                      tag=f"kn{b}", bufs=2)
                    # the sync queue is idle mid-chunk; keeping these off the
                    # scalar queue avoids head-of-line blocking of the x loads
                    eng = nc.sync
                    for h in range(HPC):
                        for sub in range(2):
                            eng.dma_start_transpose(
                                kn_s[:, 2 * h + sub, :],
                                kd_s[:, h, t0 + sub * P:t0 + (sub + 1) * P])
                    kn_l.append(kn_s)

                proj(wg_t, AF.Sigmoid, g_s)
                for t2 in range(4):
                    ps = psum.tile([P, CH], FP32, tag="ps512", bufs=4)
                    for hc in range(HC):
                        nc.tensor.matmul(
                            ps[:], xsl(hc)[:, t2 * P:(t2 + 1) * P],
                            wv_t[:, hc, :],
                            start=(hc == 0), stop=(hc == HC - 1))
                    nc.scalar.activation(v_s[:, t2, :], ps[:], AF.Silu)

                # ---- attention blocks ----
                for b in range(NBLK):
                    t0 = b * BLOCK
                    kn_s = kn_l[b]

                    # scores (transposed): sT[j, i] = k_j . q_i. The j-high
                    # half only attends to i >= 128, so its matmul and mask
                    # are half-width.
                    pss = []
                    for h in range(HPC):
                        ps_s = psum.tile([P, BLOCK + P], FP32, tag="psS", bufs=2)
                        nc.tensor.matmul(
                            ps_s[:, :BLOCK], k_s[:, h, t0:t0 + P],
                            q_s[:, h, t0:t0 + BLOCK], start=True, stop=True)
                        nc.tensor.matmul(
                            ps_s[:, BLOCK:], k_s[:, h, t0 + P:t0 + BLOCK],
                            q_s[:, h, t0 + P:t0 + BLOCK], start=True, stop=True)
                        pss.append(ps_s)

                    # decay masks + q-decay on DVE while PE streams
                    s0l, s1l, qdl = [], [], []
                    for h in range(HPC):
                        s0 = spool.tile([P, BLOCK], BF16, tag="s0", bufs=4)
                        nc.vector.tensor_mul(s0[:], pss[h][:, :BLOCK],
                                             dmask0_t[:, h, :])
                        s1 = spool.tile([P, P], BF16, tag="s1", bufs=4)
                        nc.vector.tensor_mul(s1[:], pss[h][:, BLOCK:],
                                             dmask1_t[:, h, :])
                        qd = spool.tile([P, BLOCK], BF16, tag="qd", bufs=4)
                        nc.vector.tensor_mul(qd[:], q_s[:, h, t0:t0 + BLOCK],
                                             qdec_t[:, h, :])
                        s0l.append(s0); s1l.append(s1); qdl.append(qd)

                    # attention output (transposed) + kv update
                    psal = []
                    for h in range(HPC):
                        hsl = slice(h * P, (h + 1) * P)
                        psa = psum.tile([P, BLOCK + P], FP32, tag="psA", bufs=2)
                        # o = qdec*(kv^T q) + v0^T s0 + v1^T s1   [e, i]
                        nc.tensor.matmul(psa[:, :BLOCK], kv_t[:, h, :], qdl[h][:],
                                         start=True, stop=False,
                                         skip_group_check=True)
                        nc.tensor.matmul(psa[:, :BLOCK], v_s[:, 2 * b, hsl],
                                         s0l[h][:], start=False, stop=False,
                                         skip_group_check=True)
                        nc.tensor.matmul(psa[:, P:BLOCK], v_s[:, 2 * b + 1, hsl],
                                         s1l[h][:], start=False, stop=True,
                                         skip_group_check=True)
                        # kv' = bd*kv + kn0^T v0 + kn1^T v1
                        nc.tensor.matmul(psa[:, BLOCK:], bdi_t[:, h, :],
                                         kv_t[:, h, :], start=True, stop=False)
                        nc.tensor.matmul(psa[:, BLOCK:], kn_s[:, 2 * h, :],
                                         v_s[:, 2 * b, hsl], start=False, stop=False)
                        nc.tensor.matmul(psa[:, BLOCK:], kn_s[:, 2 * h + 1, :],
                                         v_s[:, 2 * b + 1, hsl], start=False, stop=True)
                        psal.append(psa)

                    for h in range(HPC):
                        # kv state back to SBUF (bf16) for the next block
                        nc.scalar.copy(kv_t[:, h, :], psal[h][:, BLOCK:])
                        # gate multiply + squares straight from PSUM
                        nc.vector.tensor_mul(gA_s[:, h, t0:t0 + BLOCK],
                                             psal[h][:, :BLOCK],
                                             g_s[:, h, t0:t0 + BLOCK])
                        nc.scalar.square(sq_s[:, h, t0:t0 + BLOCK],
                                         psal[h][:, :BLOCK])

                # ---- out projection: pout[t] = gA @ wo ----
                for m in range(4):
                    ob = spool.tile([P, HIDDEN], BF16, tag="ob", bufs=2)
                    for ntc in range(4):
                        ps = psum.tile([P, CH], FP32, tag="ps512", bufs=4)
                        for h in range(HPC):
                            nc.tensor.matmul(
                                ps[:], gA_s[:, h, m * P:(m + 1) * P],
                                wo_t[:, h, ntc * CH:(ntc + 1) * CH],
                                start=(h == 0), stop=(h == HPC - 1))
                        nc.vector.tensor_copy(
                            out=ob[:, ntc * CH:(ntc + 1) * CH], in_=ps[:])
                    nc.sync.dma_start(
                        pout_d[t * CH + m * P:t * CH + (m + 1) * P, :], ob[:])

                # ---- ssq = sum over this core's channels of attn^2 ----
                ps = psum.tile([P, CH], FP32, tag="ps512", bufs=4)
                for h in range(HPC):
                    nc.tensor.matmul(ps[:1, :], ones_t[:], sq_s[:, h, :],
                                     start=(h == 0), stop=(h == HPC - 1))
                ssb = spool.tile([1, CH], FP32, tag="ssb", bufs=2)
                nc.scalar.copy(ssb[:], ps[:1, :])
                nc.sync.dma_start(ssq_d[:, tsl], ssb[:])

    nc.compile()
    return nc


_NC_CACHE = {}


def _get_nc(repeat=1, phases="AB", nA=1, nB=1):
    key = (repeat, phases, nA, nB)
    if key not in _NC_CACHE:
        _NC_CACHE[key] = build_nc(repeat, phases, nA, nB)
    return _NC_CACHE[key]


def make_in_maps(inputs):
    hs = np.asarray(inputs["hidden_states"], dtype=np.float32)
    w_qkv = np.asarray(inputs["w_qkv"], dtype=np.float32)
    w_gate = np.asarray(inputs["w_gate"], dtype=np.float32)
    w_out = np.asarray(inputs["w_out"], dtype=np.float32)
    norm_weight = np.asarray(inputs["norm_weight"], dtype=np.float32)
    slope_rate = np.asarray(inputs["slope_rate"], dtype=np.float32).reshape(NUM_HEADS)
    kv_cache = np.asarray(inputs["kv_cache"], dtype=np.float32)

    xt = np.ascontiguousarray(hs.T).astype(BF)          # [HIDDEN, SEQ]
    wq3 = w_qkv.reshape(HIDDEN, NUM_HEADS, 3 * HEAD_DIM)
    # norm_weight folded into w_out rows
    w_out_n = w_out * norm_weight[:, None]
    ones = np.ones((P, 1), dtype=BF)
    idx = np.arange(BLOCK, dtype=np.float64)

    in_maps = []
    for c in range(N_CORES):
        s = slope_rate[c * HPC:(c + 1) * HPC].astype(np.float64)  # [HPC]
        wq = np.ascontiguousarray(
            wq3[:, c * HPC:(c + 1) * HPC, 0:HEAD_DIM].reshape(HIDDEN, IN_PC))
        wk = np.ascontiguousarray(
            wq3[:, c * HPC:(c + 1) * HPC, HEAD_DIM:2 * HEAD_DIM].reshape(HIDDEN, IN_PC))
        wv = np.ascontiguousarray(
            wq3[:, c * HPC:(c + 1) * HPC, 2 * HEAD_DIM:3 * HEAD_DIM].reshape(HIDDEN, IN_PC))
        wg = np.ascontiguousarray(w_gate[:, c * IN_PC:(c + 1) * IN_PC])
        wo = np.ascontiguousarray(w_out_n[c * IN_PC:(c + 1) * IN_PC, :])

        # dmask0[h, j, i] = exp(-s (i - j)) for i >= j (j in 0..127, i in 0..255)
        jj = idx[:128][:, None]                          # [128,1]
        ii = idx[None, :]                                # [1,256]
        d0 = np.exp(-s[:, None, None] * (ii - jj)) * (ii >= jj)
        dmask0 = d0.astype(BF)                           # [HPC,128,256]
        # dmask1[h, j', i'] for abs j = j'+128, abs i = i'+128: dmask0[j', i']
        dmask1 = np.ascontiguousarray(dmask0[:, :, :P])
        qdec = np.broadcast_to(
            np.exp(-s[:, None] * (idx[None, :] + 1.0))[:, None, :],
            (HPC, P, BLOCK)).astype(BF)
        kdec = np.broadcast_to(
            np.exp(-s[:, None] * (BLOCK - 1.0 - idx[None, :]))[:, None, :],
            (HPC, P, BLOCK)).astype(BF)                  # [HPC, P, 256]
        bd = np.exp(-s * BLOCK)                          # [HPC]
        bdi = (bd[:, None, None] * np.eye(P)[None]).astype(BF)
        kv0 = np.ascontiguousarray(kv_cache[c * HPC:(c + 1) * HPC]).astype(BF)

        in_maps.append({
            "xt": xt, "wq": wq.astype(BF), "wk": wk.astype(BF),
            "wv": wv.astype(BF), "wg": wg.astype(BF), "wo": wo.astype(BF),
            "qdec": np.ascontiguousarray(qdec),
            "dmask0": np.ascontiguousarray(dmask0), "dmask1": dmask1,
            "kdec": np.ascontiguousarray(kdec), "bdi": bdi, "ones": ones,
            "kv0": kv0,
        })
    return in_maps


def combine_outputs(results):
    pout = np.zeros((SEQ, HIDDEN), dtype=np.float64)
    ssq = np.zeros((SEQ,), dtype=np.float64)
    for r in results:
        pout += r["pout"].astype(np.float64)
        ssq += r["ssq"].reshape(SEQ).astype(np.float64)
    var = ssq / INNER
    scale = 1.0 / np.sqrt(var + EPS)
    return (pout * scale[:, None]).astype(np.float32)


def kernel(**inputs):
    nc = _get_nc(1)
    in_maps = make_in_maps(inputs)
    res = run_bass_kernel_spmd(nc, in_maps, core_ids=list(range(N_CORES)))
    return combine_outputs(res.results)


# revision 31
# speedup vs baseline: 570.9150x; 570.9150x over previous
"""MiniMaxText01 linear attention layer on 8 trn2 NeuronCores.

Strategy: tensor-parallel over heads (4 heads/core), single fused pass in
bf16 (fp32 PSUM accumulation everywhere). Per 512-token chunk, entirely
SBUF-resident (no DRAM staging roundtrip):
  1. q/k/gate/v projections (bf16 matmuls, silu/sigmoid on ACT writing
     bf16 staging). q/k/gate staged transposed [d, tok], v natural
     [tok, c]. x chunks stream on the Activation HWDGE queue, weights on
     the SP queue, so neither delays the other.
  2. blocked lightning attention (BLOCK=256): scores = kT-slices @ q with
     decay+causal mask folded into one DVE multiply (the j>=128 half only
     attends to i>=128, so it is half-width); k natural obtained via XBAR
     DMA transposes of k*kdec (DVE) issued a projection early; kv state
     [d, 4h, e] bf16 in SBUF, decay bd*kv folded into the kv-update PSUM
     accumulation as a (bd*I) matmul.
  3. gate multiply (DVE, reading PSUM), squares for the RMSNorm variance
     (ACT), ssq channel-reduction via a ones-vector matmul, out
     projection (bf16) copied PSUM->SBUF on DVE and DMAed out as bf16.
norm_weight is folded into w_out rows on the host; the RMSNorm rsqrt is a
per-token scalar applied on the host after summing partial outputs:
  out = sum_c(pout_c) * rsqrt(sum_c(ssq_c)/4096 + eps).
"""
import numpy as np
from contextlib import ExitStack

import ml_dtypes
import concourse.bass as bass
import concourse.tile as tile
import concourse.mybir as mybir
from concourse import bacc
from concourse.bass_utils import run_bass_kernel_spmd

FP32 = mybir.dt.float32
BF16 = mybir.dt.bfloat16
AF = mybir.ActivationFunctionType
BF = ml_dtypes.bfloat16

SEQ = 4096
HIDDEN = 2048
NUM_HEADS = 32
HEAD_DIM = 128
INNER = NUM_HEADS * HEAD_DIM
BLOCK = 256
EPS = 1e-5
N_CORES = 8
HPC = NUM_HEADS // N_CORES          # 4 heads per core
IN_PC = HPC * HEAD_DIM              # 512 inner channels per core
P = 128

CH = 512                            # token chunk
NT = SEQ // CH                      # 8 chunks
HC = HIDDEN // P                    # 16 hidden chunks
NBLK = CH // BLOCK                  # 2 blocks per chunk


def build_nc(repeat: int = 1, phases: str = "AB", nA: int = 1, nB: int = 1):
    nc = bacc.Bacc("TRN2", target_bir_lowering=False)

    xt_d = nc.dram_tensor("xt", [HIDDEN, SEQ], BF16, kind="ExternalInput")
    wq_d = nc.dram_tensor("wq", [HIDDEN, IN_PC], BF16, kind="ExternalInput")
    wk_d = nc.dram_tensor("wk", [HIDDEN, IN_PC], BF16, kind="ExternalInput")
    wv_d = nc.dram_tensor("wv", [HIDDEN, IN_PC], BF16, kind="ExternalInput")
    wg_d = nc.dram_tensor("wg", [HIDDEN, IN_PC], BF16, kind="ExternalInput")
    wo_d = nc.dram_tensor("wo", [IN_PC, HIDDEN], BF16, kind="ExternalInput")
    qdec_d = nc.dram_tensor("qdec", [HPC, P, BLOCK], BF16, kind="ExternalInput")
    dmask0_d = nc.dram_tensor("dmask0", [HPC, P, BLOCK], BF16, kind="ExternalInput")
    dmask1_d = nc.dram_tensor("dmask1", [HPC, P, P], BF16, kind="ExternalInput")
    kdec_d = nc.dram_tensor("kdec", [HPC, P, BLOCK], BF16, kind="ExternalInput")
    bdi_d = nc.dram_tensor("bdi", [HPC, P, P], BF16, kind="ExternalInput")
    ones_d = nc.dram_tensor("ones", [P, 1], BF16, kind="ExternalInput")
    kv0_d = nc.dram_tensor("kv0", [HPC, P, P], BF16, kind="ExternalInput")

    pout_d = nc.dram_tensor("pout", [SEQ, HIDDEN], BF16, kind="ExternalOutput")
    ssq_d = nc.dram_tensor("ssq", [1, SEQ], FP32, kind="ExternalOutput")

    with tile.TileContext(nc) as tc, ExitStack() as ctx:
        const = ctx.enter_context(tc.tile_pool(name="const", bufs=1))
        wpool = ctx.enter_context(tc.tile_pool(name="w", bufs=1))
        xpool = ctx.enter_context(tc.tile_pool(name="x", bufs=1))
        spool = ctx.enter_context(tc.tile_pool(name="stg", bufs=1))
        psum = ctx.enter_context(tc.tile_pool(name="psum", bufs=1, space="PSUM"))

        # ---- weights first on the sync queue (q -> k -> g -> v), so the
        # first projection can start as soon as wq lands; consts + wo after.
        WG = 4                          # hc-groups per weight DMA
        wq_t = wpool.tile([P, HC, IN_PC], BF16)
        for g in range(HC // WG):
            nc.sync.dma_start(
                wq_t[:, g * WG:(g + 1) * WG, :],
                wq_d[g * WG * P:(g + 1) * WG * P, :]
                .rearrange("(hc p) m -> p hc m", p=P))
        wq_l = wq_t
        wk_t = wpool.tile([P, HC, IN_PC], BF16)
        wg_t = wpool.tile([P, HC, IN_PC], BF16)
        wv_t = wpool.tile([P, HC, IN_PC], BF16)
        for w_t, w_d in ((wk_t, wk_d), (wg_t, wg_d), (wv_t, wv_d)):
            for g in range(HC // WG):
                hsl = slice(g * WG * P, (g + 1) * WG * P)
                gsl = slice(g * WG, (g + 1) * WG)
                nc.sync.dma_start(
                    w_t[:, gsl, :],
                    w_d[hsl, :].rearrange("(hc p) m -> p hc m", p=P))

        ones_t = const.tile([P, 1], BF16)
        nc.sync.dma_start(ones_t[:], ones_d[:])
        qdec_t = const.tile([P, HPC, BLOCK], BF16)
        nc.sync.dma_start(qdec_t[:], qdec_d[:].rearrange("h p i -> p h i"))
        dmask0_t = const.tile([P, HPC, BLOCK], BF16)
        nc.sync.dma_start(dmask0_t[:], dmask0_d[:].rearrange("h p i -> p h i"))
        dmask1_t = const.tile([P, HPC, P], BF16)
        nc.sync.dma_start(dmask1_t[:], dmask1_d[:].rearrange("h p i -> p h i"))
        kdec_t = const.tile([P, HPC, BLOCK], BF16)
        nc.sync.dma_start(kdec_t[:], kdec_d[:].rearrange("h p i -> p h i"))
        bdi_t = const.tile([P, HPC, P], BF16)
        nc.sync.dma_start(bdi_t[:], bdi_d[:].rearrange("h d e -> d h e"))
        kv_t = const.tile([P, HPC, P], BF16)

        wo_t = wpool.tile([P, HPC, HIDDEN], BF16)
        nc.sync.dma_start(wo_t[:], wo_d[:].rearrange("(h p) n -> p h n", p=P))

        xt_r = xt_d[:].rearrange("(hc p) n -> p hc n", p=P)

        for _rep in range(repeat):
            nc.scalar.dma_start(kv_t[:], kv0_d[:].rearrange("h d e -> d h e"))

            for t in range(NT):
                tsl = slice(t * CH, (t + 1) * CH)

                # x chunk on the Activation HWDGE queue so weight/pout
                # traffic on the SP queue never delays it; 4 separate tiles
                # so early hc-slices are consumable while the rest stream in
                xs_l = []
                for g in range(4):
                    xg = xpool.tile([P, 4, CH], BF16, tag=f"x{g}", bufs=2)
                    nc.scalar.dma_start(xg[:], xt_r[:, g * 4:(g + 1) * 4, tsl])
                    xs_l.append(xg)

                def xsl(hc):
                    return xs_l[hc // 4][:, hc % 4, :]

                # ---- projections ----
                q_s = spool.tile([P, HPC, CH], BF16, tag="q", bufs=2)
                k_s = spool.tile([P, HPC, CH], BF16, tag="k", bufs=2)
                g_s = spool.tile([P, HPC, CH], BF16, tag="g", bufs=2)
                v_s = spool.tile([P, NBLK * 2, IN_PC], BF16, tag="v", bufs=2)

                def proj(w_l, actf, dst):
                    for cc in range(HPC):
                        ps = psum.tile([P, CH], FP32, tag="ps512", bufs=4)
                        for hc in range(HC):
                            w_t = w_l[hc // 4] if isinstance(w_l, list) else w_l
                            wi = hc % 4 if isinstance(w_l, list) else hc
                            nc.tensor.matmul(
                                ps[:], w_t[:, wi, cc * P:(cc + 1) * P],
                                xsl(hc),
                                start=(hc == 0), stop=(hc == HC - 1))
                        nc.scalar.activation(dst[:, cc, :], ps[:], actf)

                proj(wq_l, AF.Silu, q_s)
                proj(wk_t, AF.Silu, k_s)

                gA_s = spool.tile([P, HPC, CH], BF16, tag="gA", bufs=2)
                sq_s = spool.tile([P, HPC, CH], BF16, tag="sq", bufs=2)

                # k * k-decay (DVE), then k natural via XBAR DMA transpose —
                # emitted right after the k projection so the DMA latency
                # hides under the g/v projection matmuls.
                kd_s = spool.tile([P, HPC, CH], BF16, tag="kd", bufs=2)
                kn_l = []
                for b in range(NBLK):
                    t0 = b * BLOCK
                    for h in range(HPC):
                        nc.vector.tensor_mul(kd_s[:, h, t0:t0 + BLOCK],
                                             k_s[:, h, t0:t0 + BLOCK],
                                             kdec_t[:, h, :])
                    kn_s = spool.tile([P, 2 * HPC, P], BF16,
                                      tag=f"kn{b}", bufs=2)
                    # the sync queue is idle mid-chunk; keeping these off the
                    # scalar queue avoids head-of-line blocking of the x loads
                    eng = nc.sync
                    for h in range(HPC):
                        for sub in range(2):
                            eng.dma_start_transpose(
                                kn_s[:, 2 * h + sub, :],
                                kd_s[:, h, t0 + sub * P:t0 + (sub + 1) * P])
                    kn_l.append(kn_s)

                proj(wg_t, AF.Sigmoid, g_s)
                for t2 in range(4):
                    ps = psum.tile([P, CH], FP32, tag="ps512", bufs=4)
                    for hc in range(HC):
                        nc.tensor.matmul(
                            ps[:], xsl(hc)[:, t2 * P:(t2 + 1) * P],
                            wv_t[:, hc, :],
                            start=(hc == 0), stop=(hc == HC - 1))
                    nc.scalar.activation(v_s[:, t2, :], ps[:], AF.Silu)

                # ---- attention blocks ----
                for b in range(NBLK):
                    t0 = b * BLOCK
                    kn_s = kn_l[b]

                    # scores (transposed): sT[j, i] = k_j . q_i. The j-high
                    # half only attends to i >= 128, so its matmul and mask
                    # are half-width.
                    pss = []
                    for h in range(HPC):
                        ps_s = psum.tile([P, BLOCK + P], FP32, tag="psS", bufs=2)
                        nc.tensor.matmul(
                            ps_s[:, :BLOCK], k_s[:, h, t0:t0 + P],
                            q_s[:, h, t0:t0 + BLOCK], start=True, stop=True)
                        nc.tensor.matmul(
                            ps_s[:, BLOCK:], k_s[:, h, t0 + P:t0 + BLOCK],
                            q_s[:, h, t0 + P:t0 + BLOCK], start=True, stop=True)
                        pss.append(ps_s)

                    # decay masks + q-decay on DVE while PE streams
                    s0l, s1l, qdl = [], [], []
                    for h in range(HPC):
                        s0 = spool.tile([P, BLOCK], BF16, tag="s0", bufs=4)
                        nc.vector.tensor_mul(s0[:], pss[h][:, :BLOCK],
                                             dmask0_t[:, h, :])
                        s1 = spool.tile([P, P], BF16, tag="s1", bufs=4)
                        nc.vector.tensor_mul(s1[:], pss[h][:, BLOCK:],
                                             dmask1_t[:, h, :])
                        qd = spool.tile([P, BLOCK], BF16, tag="qd", bufs=4)
                        nc.vector.tensor_mul(qd[:], q_s[:, h, t0:t0 + BLOCK],
                                             qdec_t[:, h, :])
                        s0l.append(s0); s1l.append(s1); qdl.append(qd)

                    # attention output (transposed) + kv update
                    psal = []
                    for h in range(HPC):
                        hsl = slice(h * P, (h + 1) * P)
                        psa = psum.tile([P, BLOCK + P], FP32, tag="psA", bufs=2)
                        # o = qdec*(kv^T q) + v0^T s0 + v1^T s1   [e, i]
                        nc.tensor.matmul(psa[:, :BLOCK], kv_t[:, h, :], qdl[h][:],
                                         start=True, stop=False,
                                         skip_group_check=True)
                        nc.tensor.matmul(psa[:, :BLOCK], v_s[:, 2 * b, hsl],
                                         s0l[h][:], start=False, stop=False,
                                         skip_group_check=True)
                        nc.tensor.matmul(psa[:, P:BLOCK], v_s[:, 2 * b + 1, hsl],
                                         s1l[h][:], start=False, stop=True,
                                         skip_group_check=True)
                        # kv' = bd*kv + kn0^T v0 + kn1^T v1
                        nc.tensor.matmul(psa[:, BLOCK:], bdi_t[:, h, :],
                                         kv_t[:, h, :], start=True, stop=False)
                        nc.tensor.matmul(psa[:, BLOCK:], kn_s[:, 2 * h, :],
                                         v_s[:, 2 * b, hsl], start=False, stop=False)
                        nc.tensor.matmul(psa[:, BLOCK:], kn_s[:, 2 * h + 1, :],
                                         v_s[:, 2 * b + 1, hsl], start=False, stop=True)
                        psal.append(psa)

                    for h in range(HPC):
                        # kv state back to SBUF (bf16) for the next block
                        nc.scalar.copy(kv_t[:, h, :], psal[h][:, BLOCK:])
                        # gate multiply + squares straight from PSUM
                        nc.vector.tensor_mul(gA_s[:, h, t0:t0 + BLOCK],
                                             psal[h][:, :BLOCK],
                                             g_s[:, h, t0:t0 + BLOCK])
                        nc.scalar.square(sq_s[:, h, t0:t0 + BLOCK],
                                         psal[h][:, :BLOCK])

                # ---- out projection: pout[t] = gA @ wo ----
                for m in range(4):
                    ob = spool.tile([P, HIDDEN], BF16, tag="ob", bufs=2)
                    for ntc in range(4):
                        ps = psum.tile([P, CH], FP32, tag="ps512", bufs=4)
                        for h in range(HPC):
                            nc.tensor.matmul(
                                ps[:], gA_s[:, h, m * P:(m + 1) * P],
                                wo_t[:, h, ntc * CH:(ntc + 1) * CH],
                                start=(h == 0), stop=(h == HPC - 1))
                        nc.vector.tensor_copy(
                            out=ob[:, ntc * CH:(ntc + 1) * CH], in_=ps[:])
                    nc.sync.dma_start(
                        pout_d[t * CH + m * P:t * CH + (m + 1) * P, :], ob[:])

                # ---- ssq = sum over this core's channels of attn^2 ----
                ps = psum.tile([P, CH], FP32, tag="ps512", bufs=4)
                for h in range(HPC):
                    nc.tensor.matmul(ps[:1, :], ones_t[:], sq_s[:, h, :],
                                     start=(h == 0), stop=(h == HPC - 1))
                ssb = spool.tile([1, CH], FP32, tag="ssb", bufs=2)
                nc.scalar.copy(ssb[:], ps[:1, :])
                nc.sync.dma_start(ssq_d[:, tsl], ssb[:])

    nc.compile()
    return nc


_NC_CACHE = {}


def _get_nc(repeat=1, phases="AB", nA=1, nB=1):
    key = (repeat, phases, nA, nB)
    if key not in _NC_CACHE:
        _NC_CACHE[key] = build_nc(repeat, phases, nA, nB)
    return _NC_CACHE[key]


def make_in_maps(inputs):
    hs = np.asarray(inputs["hidden_states"], dtype=np.float32)
    w_qkv = np.asarray(inputs["w_qkv"], dtype=np.float32)
    w_gate = np.asarray(inputs["w_gate"], dtype=np.float32)
    w_out = np.asarray(inputs["w_out"], dtype=np.float32)
    norm_weight = np.asarray(inputs["norm_weight"], dtype=np.float32)
    slope_rate = np.asarray(inputs["slope_rate"], dtype=np.float32).reshape(NUM_HEADS)
    kv_cache = np.asarray(inputs["kv_cache"], dtype=np.float32)

    xt = np.ascontiguousarray(hs.T).astype(BF)          # [HIDDEN, SEQ]
    wq3 = w_qkv.reshape(HIDDEN, NUM_HEADS, 3 * HEAD_DIM)
    # norm_weight folded into w_out rows
    w_out_n = w_out * norm_weight[:, None]
    ones = np.ones((P, 1), dtype=BF)
    idx = np.arange(BLOCK, dtype=np.float64)

    in_maps = []
    for c in range(N_CORES):
        s = slope_rate[c * HPC:(c + 1) * HPC].astype(np.float64)  # [HPC]
        wq = np.ascontiguousarray(
            wq3[:, c * HPC:(c + 1) * HPC, 0:HEAD_DIM].reshape(HIDDEN, IN_PC))
        wk = np.ascontiguousarray(
            wq3[:, c * HPC:(c + 1) * HPC, HEAD_DIM:2 * HEAD_DIM].reshape(HIDDEN, IN_PC))
        wv = np.ascontiguousarray(
            wq3[:, c * HPC:(c + 1) * HPC, 2 * HEAD_DIM:3 * HEAD_DIM].reshape(HIDDEN, IN_PC))
        wg = np.ascontiguousarray(w_gate[:, c * IN_PC:(c + 1) * IN_PC])
        wo = np.ascontiguousarray(w_out_n[c * IN_PC:(c + 1) * IN_PC, :])

        # dmask0[h, j, i] = exp(-s (i - j)) for i >= j (j in 0..127, i in 0..255)
        jj = idx[:128][:, None]                          # [128,1]
        ii = idx[None, :]                                # [1,256]
        d0 = np.exp(-s[:, None, None] * (ii - jj)) * (ii >= jj)
        dmask0 = d0.astype(BF)                           # [HPC,128,256]
        # dmask1[h, j', i'] for abs j = j'+128, abs i = i'+128: dmask0[j', i']
        dmask1 = np.ascontiguousarray(dmask0[:, :, :P])
        qdec = np.broadcast_to(
            np.exp(-s[:, None] * (idx[None, :] + 1.0))[:, None, :],
            (HPC, P, BLOCK)).astype(BF)
        kdec = np.broadcast_to(
            np.exp(-s[:, None] * (BLOCK - 1.0 - idx[None, :]))[:, None, :],
            (HPC, P, BLOCK)).astype(BF)                  # [HPC, P, 256]
        bd = np.exp(-s * BLOCK)                          # [HPC]
        bdi = (bd[:, None, None] * np.eye(P)[None]).astype(BF)
        kv0 = np.ascontiguousarray(kv_cache[c * HPC:(c + 1) * HPC]).astype(BF)

        in_maps.append({
            "xt": xt, "wq": wq.astype(BF), "wk": wk.astype(BF),
            "wv": wv.astype(BF), "wg": wg.astype(BF), "wo": wo.astype(BF),
            "qdec": np.ascontiguousarray(qdec),
            "dmask0": np.ascontiguousarray(dmask0), "dmask1": dmask1,
            "kdec": np.ascontiguousarray(kdec), "bdi": bdi, "ones": ones,
            "kv0": kv0,
        })
    return in_maps


def combine_outputs(results):
    pout = np.zeros((SEQ, HIDDEN), dtype=np.float64)
    ssq = np.zeros((SEQ,), dtype=np.float64)
    for r in results:
        pout += r["pout"].astype(np.float64)
        ssq += r["ssq"].reshape(SEQ).astype(np.float64)
    var = ssq / INNER
    scale = 1.0 / np.sqrt(var + EPS)
    return (pout * scale[:, None]).astype(np.float32)


def kernel(**inputs):
    nc = _get_nc(1)
    in_maps = make_in_maps(inputs)
    res = run_bass_kernel_spmd(nc, in_maps, core_ids=list(range(N_CORES)))
    return combine_outputs(res.results)
